# revision 1
# baseline (speedup 1.0000x reference)
"""Trainium2 Bass kernel for nn_BaseRNN (2-layer masked tanh RNN + sigmoid head).

Strategy (data-parallel over 8 NeuronCores, 32 batch rows per core):
  - Embedding rows gathered on-device by indirect DMA (bf16 table), PE-transposed
    to [E, tokens] layout.
  - Input projection x @ W_ih computed as one 512-wide matmul per 16-step group
    directly into a PSUM bank.
  - Recurrence h[t] = tanh(xproj[t] + h[t-1] @ W_hh + b): per step one
    matmul-accumulate into the PSUM slice + one ACT tanh (bias fused) back to SBUF.
  - The two layers run as two pipelined chains, layer 2 one group behind layer 1.
  - Packed-sequence masking: final hidden captured per step with copy_predicated
    against a host-precomputed (lengths-1 == t) mask.
  - Sigmoid classifier evaluated on-device; output assembled on host.
"""

import sys

sys.path.insert(0, "/opt/trn_rl_repo")

import numpy as np

import bass_rust
import concourse.bass as bass
import concourse.tile as tile
from concourse import mybir
from concourse.bass import IndirectOffsetOnAxis
from concourse.bass_utils import run_bass_kernel_spmd
from concourse.masks import make_identity

BF16 = mybir.dt.bfloat16
F32 = mybir.dt.float32
I32 = mybir.dt.int32
NP_BF16 = mybir.dt.np(BF16)

VOCAB = 50000
E = 128
H = 128
N_CORES = 8
GS = 16  # steps per group (one PSUM bank = 512 fp32 = 16 * 32 columns)
RING = 64  # h2 ring slots

_counter = [0]


def _split_excess_waits(nc, max_waits=1):
    """walrus in this container rejects >1 semaphore wait per instruction
    ("Too many sync wait commands"). Move excess waits onto preceding NoOps on
    the same engine; in-order engines block identically."""
    n = 0
    for f in nc.m.functions:
        for bb in f.blocks:
            insts = bb.instructions
            i = 0
            while i < len(insts):
                inst = insts[i]
                si = inst.sync_info
                if si is not None and len(si.on_wait) > max_waits:
                    waits = list(si.on_wait)
                    keep, excess = waits[-max_waits:], waits[:-max_waits]
                    nops = []
                    for w in excess:
                        _counter[0] += 1
                        nop = mybir.InstNoOp(
                            name=f"wsplit_{_counter[0]}", engine=inst.engine
                        )
                        nop.sync_info = bass_rust.SyncInfo(on_wait=[w], on_update=[])
                        nops.append(nop)
                    inst.sync_info = bass_rust.SyncInfo(
                        on_wait=keep, on_update=list(si.on_update)
                    )
                    insts[i:i] = nops
                    i += len(nops)
                    n += 1
                i += 1
    return n


def build_program(T, BL):
    """Build the SPMD Bass program for seq length T and BL batch rows/core."""
    assert T % GS == 0 and BL == 32
    groups = T // GS
    n_tok = T * BL  # tokens per core
    chunks = (GS * BL) // 128  # 128-row gather chunks per group (=4)

    nc = bass.Bass()

    tok_d = nc.declare_dram_parameter("tok", [128, chunks * groups], I32, isOutput=False)
    emb_d = nc.declare_dram_parameter("emb", [VOCAB, E], BF16, isOutput=False)
    w_d = nc.declare_dram_parameter("w", [128, 512], BF16, isOutput=False)
    b1_d = nc.declare_dram_parameter("b1", [128, 1], F32, isOutput=False)
    b2_d = nc.declare_dram_parameter("b2", [128, 1], F32, isOutput=False)
    clsw_d = nc.declare_dram_parameter("clsw", [128, 1], BF16, isOutput=False)
    clsb_d = nc.declare_dram_parameter("clsb", [1, 1], F32, isOutput=False)
    eqm_d = nc.declare_dram_parameter("eqm", [128, n_tok], mybir.dt.uint8, isOutput=False)
    out_d = nc.declare_dram_parameter("out", [1, BL], F32, isOutput=True)

    with tile.TileContext(nc) as tc:
        with (
            tc.tile_pool(name="const", bufs=1) as const,
            tc.tile_pool(name="exg", bufs=3) as exgp,
            tc.tile_pool(name="ext", bufs=3) as extp,
            tc.tile_pool(name="mk", bufs=3) as mkp,
            tc.tile_pool(name="p1", bufs=2, space="PSUM") as p1p,
            tc.tile_pool(name="p2", bufs=2, space="PSUM") as p2p,
            tc.tile_pool(name="tp", bufs=2, space="PSUM") as tpp,
            tc.tile_pool(name="pc", bufs=1, space="PSUM") as pcp,
        ):
            # ---- persistent tiles ----
            tok_sb = const.tile([128, chunks * groups], I32)
            w_sb = const.tile([128, 512], BF16)
            b1_sb = const.tile([128, 1], F32)
            b2_sb = const.tile([128, 1], F32)
            clsw_sb = const.tile([128, 1], BF16)
            clsb_sb = const.tile([1, 1], F32)
            ident = const.tile([128, 128], BF16)
            h1h = const.tile([128, (T + 1) * BL], BF16)  # block t+1 = h1[t]
            h2r = const.tile([128, RING * BL], BF16)  # slot (t+1)%RING = h2[t]
            cap = const.tile([128, BL], BF16)
            osb = const.tile([1, BL], F32)

            nc.sync.dma_start(out=tok_sb[:], in_=tok_d[:])
            nc.sync.dma_start(out=w_sb[:], in_=w_d[:])
            nc.sync.dma_start(out=b1_sb[:], in_=b1_d[:])
            nc.sync.dma_start(out=b2_sb[:], in_=b2_d[:])
            nc.sync.dma_start(out=clsw_sb[:], in_=clsw_d[:])
            nc.sync.dma_start(out=clsb_sb[:], in_=clsb_d[:])
            make_identity(nc, ident[:])
            nc.gpsimd.memset(h1h[:, 0 : BL], 0.0)
            nc.gpsimd.memset(h2r[:, 0 : BL], 0.0)
            nc.gpsimd.memset(cap[:], 0.0)

            W_IH1 = w_sb[:, 0:128]
            W_HH1 = w_sb[:, 128:256]
            W_IH2 = w_sb[:, 256:384]
            W_HH2 = w_sb[:, 384:512]

            def h1_at(t):  # h1 state after step t; t=-1 is the zero block
                return h1h[:, (t + 1) * BL : (t + 2) * BL]

            def h2_at(t):
                s = (t + 1) % RING
                return h2r[:, s * BL : (s + 1) * BL]

            mk_tiles = {}

            def emit_layer1(g):
                # gather 512 embedding rows for this group
                exg = exgp.tile([128, GS * BL], BF16)
                for j in range(chunks):
                    nc.gpsimd.indirect_dma_start(
                        out=exg[:, 128 * j : 128 * (j + 1)],
                        out_offset=None,
                        in_=emb_d[:, :],
                        in_offset=IndirectOffsetOnAxis(
                            ap=tok_sb[:, chunks * g + j : chunks * g + j + 1], axis=0
                        ),
                    )
                # transpose each 128-token chunk: exT[:, c] = emb[tok_c]
                tp = tpp.tile([128, GS * BL], BF16, space="PSUM")
                for j in range(chunks):
                    sl = slice(128 * j, 128 * (j + 1))
                    nc.tensor.transpose(out=tp[:, sl], in_=exg[:, sl], identity=ident[:])
                exT = extp.tile([128, GS * BL], BF16)
                nc.vector.tensor_copy(out=exT[:], in_=tp[:])
                # stage the capture mask for this group (used by layer 2 later)
                mk = mkp.tile([128, GS * BL], mybir.dt.uint8)
                nc.sync.dma_start(
                    out=mk[:], in_=eqm_d[:, GS * BL * g : GS * BL * (g + 1)]
                )
                mk_tiles[g] = mk
                # input projection for the whole group into one PSUM bank
                p1 = p1p.tile([128, GS * BL], F32, space="PSUM")
                nc.tensor.matmul(
                    p1[:], lhsT=W_IH1, rhs=exT[:], start=True, stop=False,
                    skip_group_check=True,
                )
                # recurrence
                for s in range(GS):
                    t = GS * g + s
                    sl = slice(BL * s, BL * (s + 1))
                    nc.tensor.matmul(
                        p1[:, sl], lhsT=W_HH1, rhs=h1_at(t - 1),
                        start=False, stop=True, skip_group_check=True,
                    )
                    nc.scalar.activation(
                        out=h1_at(t), in_=p1[:, sl],
                        func=mybir.ActivationFunctionType.Tanh, bias=b1_sb[:, 0:1],
                    )

            def emit_layer2(g):
                mk = mk_tiles.pop(g)
                p2 = p2p.tile([128, GS * BL], F32, space="PSUM")
                nc.tensor.matmul(
                    p2[:], lhsT=W_IH2,
                    rhs=h1h[:, (GS * g + 1) * BL : (GS * (g + 1) + 1) * BL],
                    start=True, stop=False, skip_group_check=True,
                )
                for s in range(GS):
                    t = GS * g + s
                    sl = slice(BL * s, BL * (s + 1))
                    nc.tensor.matmul(
                        p2[:, sl], lhsT=W_HH2, rhs=h2_at(t - 1),
                        start=False, stop=True, skip_group_check=True,
                    )
                    nc.scalar.activation(
                        out=h2_at(t), in_=p2[:, sl],
                        func=mybir.ActivationFunctionType.Tanh, bias=b2_sb[:, 0:1],
                    )
                    nc.vector.copy_predicated(
                        out=cap[:], mask=mk[:, sl], data=h2_at(t)
                    )

            for g in range(groups + 1):
                if g < groups:
                    emit_layer1(g)
                if g >= 1:
                    emit_layer2(g - 1)

            # classifier: logits[1, BL] = cls_w.T @ cap ; sigmoid
            pc = pcp.tile([1, BL], F32, space="PSUM")
            nc.tensor.matmul(pc[:], lhsT=clsw_sb[:], rhs=cap[:], start=True, stop=True)
            nc.scalar.activation(
                out=osb[:], in_=pc[:],
                func=mybir.ActivationFunctionType.Sigmoid, bias=clsb_sb[:, 0:1],
            )
            nc.sync.dma_start(out=out_d[:], in_=osb[:])

    _split_excess_waits(nc)
    return nc


def make_core_inputs(x_c, lengths_c, emb_bf, w_pack, b1, b2, clsw_bf, clsb, T, BL):
    """Host-side prep of one core's input map. x_c [BL, T] int, lengths_c [BL]."""
    groups = T // GS
    chunks = (GS * BL) // 128
    # token index layout: flat[g, i] with i = t_loc*32 + b; chunk j = i >> 7
    A = x_c.T.reshape(groups, GS, BL).reshape(groups, GS * BL)  # [g, i]
    tok = (
        A.reshape(groups, chunks, 128).transpose(2, 0, 1).reshape(128, groups * chunks)
    )
    # column order must be [g*chunks + j]
    tok = np.ascontiguousarray(tok).astype(np.int32)
    eq = (np.arange(T)[:, None] == (lengths_c - 1)[None, :]).astype(np.uint8)  # [T, BL]
    eqm = np.broadcast_to(eq.reshape(1, T * BL), (128, T * BL))
    eqm = np.ascontiguousarray(eqm)
    return {
        "tok": tok,
        "emb": emb_bf,
        "w": w_pack,
        "b1": b1,
        "b2": b2,
        "clsw": clsw_bf,
        "clsb": clsb,
        "eqm": eqm,
    }


def prep_in_maps(np_inputs, T, BL):
    x = np.asarray(np_inputs["x"])
    lengths = np.asarray(np_inputs["lengths"])
    emb = np_inputs["emb"]
    W_ih, W_hh, b = np_inputs["W_ih"], np_inputs["W_hh"], np_inputs["b"]
    cls_w, cls_b = np_inputs["cls_w"], np_inputs["cls_b"]
    emb_bf = np.asarray(emb, np.float32).astype(NP_BF16)
    w_pack = np.concatenate(
        [W_ih[0], W_hh[0], W_ih[1], W_hh[1]], axis=1
    ).astype(NP_BF16)
    b1 = np.asarray(b[0], np.float32).reshape(128, 1)
    b2 = np.asarray(b[1], np.float32).reshape(128, 1)
    clsw_bf = np.asarray(cls_w, np.float32).astype(NP_BF16).reshape(128, 1)
    clsb = np.asarray(cls_b, np.float32).reshape(1, 1)

    in_maps = []
    for c in range(N_CORES):
        rc = slice(c * BL, (c + 1) * BL)
        in_maps.append(
            make_core_inputs(
                x[rc].astype(np.int64),
                lengths[rc].astype(np.int64),
                emb_bf, w_pack, b1, b2, clsw_bf, clsb, T, BL,
            )
        )
    return in_maps


def run(x, lengths, emb, W_ih, W_hh, b, cls_w, cls_b, T, BL, trace=False):
    x = np.asarray(x)
    B = x.shape[0]
    assert B == N_CORES * BL and x.shape[1] == T
    in_maps = prep_in_maps(
        dict(x=x, lengths=lengths, emb=emb, W_ih=W_ih, W_hh=W_hh, b=b,
             cls_w=cls_w, cls_b=cls_b),
        T, BL,
    )

    import time as _time

    _t = _time.time()
    nc = build_program(T, BL)
    print(f"[kernel] build_program: {_time.time() - _t:.1f}s", flush=True)
    _t = _time.time()
    res = run_bass_kernel_spmd(
        nc,
        in_maps,
        list(range(N_CORES)),
        trace=trace,
        trace_cores=list(range(N_CORES)) if trace else None,
    )
    print(f"[kernel] compile+exec: {_time.time() - _t:.1f}s", flush=True)
    out = np.concatenate(
        [res.results[c]["out"].reshape(BL) for c in range(N_CORES)]
    ).reshape(B, 1).astype(np.float32)
    return out, res


def kernel(x, lengths, emb, W_ih, W_hh, b, cls_w, cls_b):
    out, _ = run(x, lengths, emb, W_ih, W_hh, b, cls_w, cls_b, T=2048, BL=32)
    return out



# revision 2
# speedup vs baseline: 1.0558x; 1.0558x over previous
"""Trainium2 Bass kernel for nn_BaseRNN (2-layer masked tanh RNN + sigmoid head).

v2: step-level interleaving of the two layers (v1 serialized them per
16-step group), per-GROUP final-hidden capture instead of per-step, and
software-pipelined gather/transpose/input-proj prefetch placed into the
PE stall windows.

Strategy (data-parallel over 8 NeuronCores, 32 batch rows per core):
  - Embedding rows gathered on-device by indirect DMA (bf16 table),
    PE-transposed to [E, tokens] layout.
  - Input projection x @ W_ih computed as one 512-wide matmul per 16-step
    group directly into a PSUM bank.
  - Recurrence h[t] = tanh(xproj[t] + h[t-1] @ W_hh + b): per step one
    matmul-accumulate into the PSUM slice + one ACT tanh (bias fused) back
    to SBUF. Layer 2 runs one group behind layer 1, its per-step matmul/act
    interleaved into the same slots so the serial chains overlap.
  - Packed-sequence masking: h2 history kept in a 64-slot ring; once per
    group one copy_predicated captures rows whose (lengths-1) falls in the
    group, against a host-precomputed (lengths-1 == t) mask.
  - Sigmoid classifier evaluated on-device; output assembled on host.
"""

import sys

sys.path.insert(0, "/opt/trn_rl_repo")

import numpy as np

import bass_rust
import concourse.bass as bass
import concourse.tile as tile
from concourse import mybir
from concourse.bass import IndirectOffsetOnAxis
from concourse.bass_utils import run_bass_kernel_spmd
from concourse.masks import make_identity

BF16 = mybir.dt.bfloat16
F32 = mybir.dt.float32
I32 = mybir.dt.int32
NP_BF16 = mybir.dt.np(BF16)

VOCAB = 50000
E = 128
H = 128
N_CORES = 8
GS = 16  # steps per group (one PSUM bank = 512 fp32 = 16 * 32 columns)
RING = 64  # h2 ring slots (4 groups), slot = t % RING

_counter = [0]


def _elide_redundant_waits(nc, strip_same_engine=True, dominance=True):
    """Tile's redundant-wait optimizer is disabled in this container; do the
    sound subset ourselves: per engine, in program order, drop sem-ge-imm
    waits whose (sem id, value) is already covered by an earlier wait on the
    same engine, and merge same-instruction waits on the same sem (keep max).
    Monotonic counter semantics make value-dominance sound per engine."""
    eng_prefix = {
        mybir.EngineType.Activation: "Activation",
        mybir.EngineType.PE: "PE",
        mybir.EngineType.DVE: "DVE",
        mybir.EngineType.Pool: "Pool",
        mybir.EngineType.SP: "SP",
    }
    n = 0
    for f in nc.m.functions:
        for bb in f.blocks:
            seen = {}
            for inst in bb.instructions:
                si = inst.sync_info
                eng = inst.engine
                if si is None or not si.on_wait:
                    continue
                # Only InstActivation: acts never read what acts wrote
                # (PSUM in, SBUF out), so same-engine waits on ACT are pure
                # WAW-drain ordering, safe under FIFO write commit. PE's
                # Ldweights wait guards live weight registers; DVE's capture
                # has a genuine same-engine RAW (tmp) — keep both.
                own = (
                    eng_prefix.get(eng)
                    if strip_same_engine and type(inst).__name__ == "InstActivation"
                    else None
                )
                es = seen.setdefault(eng, {})
                merged = {}
                passthrough = []
                for w in si.on_wait:
                    if (
                        w.sync_type == "semaphore"
                        and w.wait_mode == "sem-ge-imm"
                        and w.wait_reg is None
                    ):
                        # same-engine sem wait: in-order issue + FIFO write
                        # commit already orders it; drop.
                        if own is not None and w.ant_name.rsplit("_", 1)[0] == own:
                            n += 1
                            continue
                        k = w.id
                        if k not in merged or merged[k].wait_value < w.wait_value:
                            merged[k] = w
                    else:
                        passthrough.append(w)
                kept = list(passthrough)
                for k, w in merged.items():
                    if dominance and es.get(k, -1) >= w.wait_value:
                        n += 1
                        continue
                    kept.append(w)
                    if dominance:
                        es[k] = w.wait_value
                if len(kept) != len(si.on_wait):
                    inst.sync_info = bass_rust.SyncInfo(
                        on_wait=kept, on_update=list(si.on_update)
                    )
    return n


def _split_excess_waits(nc, max_waits=1):
    """walrus in this container rejects >1 semaphore wait per instruction
    ("Too many sync wait commands"). Move excess waits onto preceding NoOps on
    the same engine; in-order engines block identically."""
    n = 0
    for f in nc.m.functions:
        for bb in f.blocks:
            insts = bb.instructions
            i = 0
            while i < len(insts):
                inst = insts[i]
                si = inst.sync_info
                if si is not None and len(si.on_wait) > max_waits:
                    waits = list(si.on_wait)
                    keep, excess = waits[-max_waits:], waits[:-max_waits]
                    nops = []
                    for w in excess:
                        _counter[0] += 1
                        nop = mybir.InstNoOp(
                            name=f"wsplit_{_counter[0]}", engine=inst.engine
                        )
                        nop.sync_info = bass_rust.SyncInfo(on_wait=[w], on_update=[])
                        nops.append(nop)
                    inst.sync_info = bass_rust.SyncInfo(
                        on_wait=keep, on_update=list(si.on_update)
                    )
                    insts[i:i] = nops
                    i += len(nops)
                    n += 1
                i += 1
    return n


def build_program(T, BL):
    """Build the SPMD Bass program for seq length T and BL batch rows/core."""
    assert T % GS == 0 and BL == 32
    groups = T // GS
    n_tok = T * BL  # tokens per core
    chunks = (GS * BL) // 128  # 128-row gather chunks per group (=4)

    nc = bass.Bass()

    tok_d = nc.declare_dram_parameter("tok", [128, chunks * groups], I32, isOutput=False)
    emb_d = nc.declare_dram_parameter("emb", [VOCAB, E], BF16, isOutput=False)
    w_d = nc.declare_dram_parameter("w", [128, 512], BF16, isOutput=False)
    b1_d = nc.declare_dram_parameter("b1", [128, 1], F32, isOutput=False)
    b2_d = nc.declare_dram_parameter("b2", [128, 1], F32, isOutput=False)
    clsw_d = nc.declare_dram_parameter("clsw", [128, 1], BF16, isOutput=False)
    clsb_d = nc.declare_dram_parameter("clsb", [1, 1], F32, isOutput=False)
    eqm_d = nc.declare_dram_parameter("eqm", [128, n_tok], BF16, isOutput=False)
    out_d = nc.declare_dram_parameter("out", [1, BL], F32, isOutput=True)

    with tile.TileContext(nc) as tc:
        with (
            tc.tile_pool(name="const", bufs=1) as const,
            tc.tile_pool(name="exg", bufs=3) as exgp,
            tc.tile_pool(name="ext", bufs=2) as extp,
            tc.tile_pool(name="mk", bufs=3) as mkp,
            tc.tile_pool(name="tmpp", bufs=2) as tmpp,
            tc.tile_pool(name="p1", bufs=2, space="PSUM") as p1p,
            tc.tile_pool(name="p2", bufs=2, space="PSUM") as p2p,
            tc.tile_pool(name="tp", bufs=2, space="PSUM") as tpp,
            tc.tile_pool(name="pc", bufs=1, space="PSUM") as pcp,
        ):
            # ---- persistent tiles ----
            tok_sb = const.tile([128, chunks * groups], I32)
            w_sb = const.tile([128, 512], BF16)
            b1_sb = const.tile([128, 1], F32)
            b2_sb = const.tile([128, 1], F32)
            clsw_sb = const.tile([128, 1], BF16)
            clsb_sb = const.tile([1, 1], F32)
            ident = const.tile([128, 128], BF16)
            h1h = const.tile([128, (T + 1) * BL], BF16)  # block t+1 = h1[t]
            h2r = const.tile([128, RING * BL], BF16)  # slot t%RING = h2[t]
            cap = const.tile([128, BL], BF16)
            capf = const.tile([128, BL], F32)
            redh = const.tile([128, BL * groups], F32)
            osb = const.tile([1, BL], F32)

            nc.sync.dma_start(out=tok_sb[:], in_=tok_d[:])
            nc.sync.dma_start(out=w_sb[:], in_=w_d[:])
            nc.sync.dma_start(out=b1_sb[:], in_=b1_d[:])
            nc.sync.dma_start(out=b2_sb[:], in_=b2_d[:])
            nc.sync.dma_start(out=clsw_sb[:], in_=clsw_d[:])
            nc.sync.dma_start(out=clsb_sb[:], in_=clsb_d[:])
            make_identity(nc, ident[:])
            nc.gpsimd.memset(h1h[:, 0:BL], 0.0)
            nc.gpsimd.memset(h2r[:, (RING - 1) * BL : RING * BL], 0.0)

            W_IH1 = w_sb[:, 0:128]
            W_HH1 = w_sb[:, 128:256]
            W_IH2 = w_sb[:, 256:384]
            W_HH2 = w_sb[:, 384:512]

            def h1_at(t):  # h1 state after step t; t=-1 is the zero block
                return h1h[:, (t + 1) * BL : (t + 2) * BL]

            def h2_at(t):  # h2 state after step t; t=-1 is ring slot RING-1
                s = t % RING
                return h2r[:, s * BL : (s + 1) * BL]

            ex_tiles = {}
            exT_tiles = {}
            mk_tiles = {}
            p1_tiles = {}
            p2_tiles = {}

            def emit_gather(g):
                exg = exgp.tile([128, GS * BL], BF16, name="exg")
                for j in range(chunks):
                    nc.gpsimd.indirect_dma_start(
                        out=exg[:, 128 * j : 128 * (j + 1)],
                        out_offset=None,
                        in_=emb_d[:, :],
                        in_offset=IndirectOffsetOnAxis(
                            ap=tok_sb[:, chunks * g + j : chunks * g + j + 1], axis=0
                        ),
                    )
                ex_tiles[g] = exg
                mk = mkp.tile([128, GS * BL], BF16, name="mk")
                nc.sync.dma_start(
                    out=mk[:], in_=eqm_d[:, GS * BL * g : GS * BL * (g + 1)]
                )
                mk_tiles[g] = mk

            def emit_transposes(g, lo, hi):
                if g not in exT_tiles:
                    # PSUM staging tile created alongside first transpose pair
                    tp_t = tpp.tile([128, GS * BL], BF16, space="PSUM", name="tp_t")
                    exT_t = extp.tile([128, GS * BL], BF16, name="exT_t")
                    exT_tiles[g] = (tp_t, exT_t)
                tp, _ = exT_tiles[g]
                exg = ex_tiles[g]
                for j in range(lo, hi):
                    sl = slice(128 * j, 128 * (j + 1))
                    nc.tensor.transpose(out=tp[:, sl], in_=exg[:, sl], identity=ident[:])

            def emit_ext_copy(g):
                tp, exT = exT_tiles[g]
                nc.vector.tensor_copy(out=exT[:], in_=tp[:])
                del ex_tiles[g]

            def emit_proj1(g):
                _, exT = exT_tiles.pop(g)
                p1 = p1p.tile([128, GS * BL], F32, space="PSUM", name="p1")
                nc.tensor.matmul(
                    p1[:], lhsT=W_IH1, rhs=exT[:], start=True, stop=False,
                    skip_group_check=True,
                )
                p1_tiles[g] = p1

            def emit_proj2(g):
                p2 = p2p.tile([128, GS * BL], F32, space="PSUM", name="p2")
                nc.tensor.matmul(
                    p2[:], lhsT=W_IH2,
                    rhs=h1h[:, (GS * g + 1) * BL : (GS * (g + 1) + 1) * BL],
                    start=True, stop=False, skip_group_check=True,
                )
                p2_tiles[g] = p2

            def emit_m1(t):
                p1 = p1_tiles[t // GS]
                s = t % GS
                nc.tensor.matmul(
                    p1[:, BL * s : BL * (s + 1)], lhsT=W_HH1, rhs=h1_at(t - 1),
                    start=False, stop=True, skip_group_check=True,
                )

            def emit_a1(t):
                p1 = p1_tiles[t // GS]
                s = t % GS
                nc.scalar.activation(
                    out=h1_at(t), in_=p1[:, BL * s : BL * (s + 1)],
                    func=mybir.ActivationFunctionType.Tanh, bias=b1_sb[:, 0:1],
                )

            def emit_m2(t):
                p2 = p2_tiles[t // GS]
                s = t % GS
                nc.tensor.matmul(
                    p2[:, BL * s : BL * (s + 1)], lhsT=W_HH2, rhs=h2_at(t - 1),
                    start=False, stop=True, skip_group_check=True,
                )

            def emit_a2(t):
                p2 = p2_tiles[t // GS]
                s = t % GS
                nc.scalar.activation(
                    out=h2_at(t), in_=p2[:, BL * s : BL * (s + 1)],
                    func=mybir.ActivationFunctionType.Tanh, bias=b2_sb[:, 0:1],
                )

            def emit_capture(g):
                mk = mk_tiles.pop(g)
                base = (g % (RING // GS)) * GS * BL
                tmp = tmpp.tile([128, GS * BL], BF16, name="cm")
                nc.vector.tensor_mul(tmp[:], h2r[:, base : base + GS * BL], mk[:])
                nc.vector.tensor_reduce(
                    out=redh[:, BL * g : BL * (g + 1)],
                    in_=tmp[:].rearrange("p (s b) -> p b s", s=GS, b=BL),
                    axis=mybir.AxisListType.X,
                    op=mybir.AluOpType.add,
                )
                del p2_tiles[g]

            # ---- prologue: stage group 0 ----
            emit_gather(0)
            emit_transposes(0, 0, chunks)
            emit_ext_copy(0)
            emit_proj1(0)

            # ---- main loop: L1 on group g, L2 on group g-1, prefetch g+1 ----
            for g in range(groups + 1):
                for s in range(GS):
                    t = GS * g + s
                    if g < groups:
                        emit_m1(t)
                    if s == 0 and g >= 1:
                        emit_proj2(g - 1)
                    if g >= 1:
                        emit_m2(GS * (g - 1) + s)
                    if g < groups:
                        emit_a1(t)
                    if g >= 1:
                        emit_a2(GS * (g - 1) + s)
                    if g + 1 < groups:
                        if s == 1:
                            emit_gather(g + 1)
                        elif s == 5:
                            emit_transposes(g + 1, 0, 2)
                        elif s == 6:
                            emit_transposes(g + 1, 2, chunks)
                        elif s == 7:
                            emit_ext_copy(g + 1)
                        elif s == 9:
                            emit_proj1(g + 1)
                if g >= 1:
                    emit_capture(g - 1)

            # fold per-group partial captures, then classifier
            nc.vector.tensor_reduce(
                out=capf[:],
                in_=redh[:].rearrange("p (g b) -> p b g", g=groups, b=BL),
                axis=mybir.AxisListType.X,
                op=mybir.AluOpType.add,
            )
            nc.vector.tensor_copy(out=cap[:], in_=capf[:])

            # classifier: logits[1, BL] = cls_w.T @ cap ; sigmoid
            pc = pcp.tile([1, BL], F32, space="PSUM")
            nc.tensor.matmul(pc[:], lhsT=clsw_sb[:], rhs=cap[:], start=True, stop=True)
            nc.scalar.activation(
                out=osb[:], in_=pc[:],
                func=mybir.ActivationFunctionType.Sigmoid, bias=clsb_sb[:, 0:1],
            )
            nc.sync.dma_start(out=out_d[:], in_=osb[:])

    import os
    # 0: none; 1: dominance only; 2: dominance + strip ACT same-engine waits;
    # 3: strip only (no dominance)
    mode = int(os.environ.get("KV2_ELIDE", "3"))
    if mode in (1, 2):
        _elide_redundant_waits(nc, strip_same_engine=(mode == 2))
    elif mode == 3:
        _elide_redundant_waits(nc, strip_same_engine=True, dominance=False)
    _split_excess_waits(nc)
    return nc


def make_core_inputs(x_c, lengths_c, emb_bf, w_pack, b1, b2, clsw_bf, clsb, T, BL):
    """Host-side prep of one core's input map. x_c [BL, T] int, lengths_c [BL]."""
    groups = T // GS
    chunks = (GS * BL) // 128
    # token index layout: flat[g, i] with i = t_loc*32 + b; chunk j = i >> 7
    A = x_c.T.reshape(groups, GS, BL).reshape(groups, GS * BL)  # [g, i]
    tok = (
        A.reshape(groups, chunks, 128).transpose(2, 0, 1).reshape(128, groups * chunks)
    )
    # column order must be [g*chunks + j]
    tok = np.ascontiguousarray(tok).astype(np.int32)
    eq = (np.arange(T)[:, None] == (lengths_c - 1)[None, :]).astype(NP_BF16)  # [T, BL]
    eqm = np.broadcast_to(eq.reshape(1, T * BL), (128, T * BL))
    eqm = np.ascontiguousarray(eqm)
    return {
        "tok": tok,
        "emb": emb_bf,
        "w": w_pack,
        "b1": b1,
        "b2": b2,
        "clsw": clsw_bf,
        "clsb": clsb,
        "eqm": eqm,
    }


def prep_in_maps(np_inputs, T, BL):
    x = np.asarray(np_inputs["x"])
    lengths = np.asarray(np_inputs["lengths"])
    emb = np_inputs["emb"]
    W_ih, W_hh, b = np_inputs["W_ih"], np_inputs["W_hh"], np_inputs["b"]
    cls_w, cls_b = np_inputs["cls_w"], np_inputs["cls_b"]
    emb_bf = np.asarray(emb, np.float32).astype(NP_BF16)
    w_pack = np.concatenate(
        [W_ih[0], W_hh[0], W_ih[1], W_hh[1]], axis=1
    ).astype(NP_BF16)
    b1 = np.asarray(b[0], np.float32).reshape(128, 1)
    b2 = np.asarray(b[1], np.float32).reshape(128, 1)
    clsw_bf = np.asarray(cls_w, np.float32).astype(NP_BF16).reshape(128, 1)
    clsb = np.asarray(cls_b, np.float32).reshape(1, 1)

    in_maps = []
    for c in range(N_CORES):
        rc = slice(c * BL, (c + 1) * BL)
        in_maps.append(
            make_core_inputs(
                x[rc].astype(np.int64),
                lengths[rc].astype(np.int64),
                emb_bf, w_pack, b1, b2, clsw_bf, clsb, T, BL,
            )
        )
    return in_maps


def run(x, lengths, emb, W_ih, W_hh, b, cls_w, cls_b, T, BL, trace=False):
    x = np.asarray(x)
    B = x.shape[0]
    assert B == N_CORES * BL and x.shape[1] == T
    in_maps = prep_in_maps(
        dict(x=x, lengths=lengths, emb=emb, W_ih=W_ih, W_hh=W_hh, b=b,
             cls_w=cls_w, cls_b=cls_b),
        T, BL,
    )

    import time as _time

    _t = _time.time()
    nc = build_program(T, BL)
    print(f"[kernel] build_program: {_time.time() - _t:.1f}s", flush=True)
    _t = _time.time()
    res = run_bass_kernel_spmd(
        nc,
        in_maps,
        list(range(N_CORES)),
        trace=trace,
        trace_cores=list(range(N_CORES)) if trace else None,
    )
    print(f"[kernel] compile+exec: {_time.time() - _t:.1f}s", flush=True)
    out = np.concatenate(
        [res.results[c]["out"].reshape(BL) for c in range(N_CORES)]
    ).reshape(B, 1).astype(np.float32)
    return out, res


def kernel(x, lengths, emb, W_ih, W_hh, b, cls_w, cls_b):
    out, _ = run(x, lengths, emb, W_ih, W_hh, b, cls_w, cls_b, T=2048, BL=32)
    return out


# revision 4
# speedup vs baseline: 1.0610x; 1.0050x over previous
"""Trainium2 Bass kernel for nn_BaseRNN (2-layer masked tanh RNN + sigmoid head).

v2: step-level interleaving of the two layers (v1 serialized them per
16-step group), per-GROUP final-hidden capture instead of per-step, and
software-pipelined gather/transpose/input-proj prefetch placed into the
PE stall windows.

Strategy (data-parallel over 8 NeuronCores, 32 batch rows per core):
  - Embedding rows gathered on-device by indirect DMA (bf16 table),
    PE-transposed to [E, tokens] layout.
  - Input projection x @ W_ih computed as one 512-wide matmul per 16-step
    group directly into a PSUM bank.
  - Recurrence h[t] = tanh(xproj[t] + h[t-1] @ W_hh + b): per step one
    matmul-accumulate into the PSUM slice + one ACT tanh (bias fused) back
    to SBUF. Layer 2 runs one group behind layer 1, its per-step matmul/act
    interleaved into the same slots so the serial chains overlap.
  - Packed-sequence masking: h2 history kept in a 64-slot ring; once per
    group one copy_predicated captures rows whose (lengths-1) falls in the
    group, against a host-precomputed (lengths-1 == t) mask.
  - Sigmoid classifier evaluated on-device; output assembled on host.
"""

import sys

sys.path.insert(0, "/opt/trn_rl_repo")

import numpy as np

import bass_rust
import concourse.bass as bass
import concourse.tile as tile
from concourse import mybir
from concourse.bass import IndirectOffsetOnAxis
from concourse.bass_utils import run_bass_kernel_spmd
from concourse.masks import make_identity

BF16 = mybir.dt.bfloat16
F32 = mybir.dt.float32
I32 = mybir.dt.int32
NP_BF16 = mybir.dt.np(BF16)

VOCAB = 50000
E = 128
H = 128
N_CORES = 8
GS = 16  # steps per group (one PSUM bank = 512 fp32 = 16 * 32 columns)
RING = 64  # h2 ring slots (4 groups), slot = t % RING

_counter = [0]


def _elide_redundant_waits(nc, strip_same_engine=True, dominance=True):
    """Tile's redundant-wait optimizer is disabled in this container; do the
    sound subset ourselves: per engine, in program order, drop sem-ge-imm
    waits whose (sem id, value) is already covered by an earlier wait on the
    same engine, and merge same-instruction waits on the same sem (keep max).
    Monotonic counter semantics make value-dominance sound per engine."""
    eng_prefix = {
        mybir.EngineType.Activation: "Activation",
        mybir.EngineType.PE: "PE",
        mybir.EngineType.DVE: "DVE",
        mybir.EngineType.Pool: "Pool",
        mybir.EngineType.SP: "SP",
    }
    n = 0
    for f in nc.m.functions:
        for bb in f.blocks:
            seen = {}
            for inst in bb.instructions:
                si = inst.sync_info
                eng = inst.engine
                if si is None or not si.on_wait:
                    continue
                # Only InstActivation: acts never read what acts wrote
                # (PSUM in, SBUF out), so same-engine waits on ACT are pure
                # WAW-drain ordering, safe under FIFO write commit. PE's
                # Ldweights wait guards live weight registers; DVE's capture
                # has a genuine same-engine RAW (tmp) — keep both.
                own = (
                    eng_prefix.get(eng)
                    if strip_same_engine and type(inst).__name__ == "InstActivation"
                    else None
                )
                es = seen.setdefault(eng, {})
                merged = {}
                passthrough = []
                for w in si.on_wait:
                    if (
                        w.sync_type == "semaphore"
                        and w.wait_mode == "sem-ge-imm"
                        and w.wait_reg is None
                    ):
                        # same-engine sem wait: in-order issue + FIFO write
                        # commit already orders it; drop.
                        if own is not None and w.ant_name.rsplit("_", 1)[0] == own:
                            n += 1
                            continue
                        k = w.id
                        if k not in merged or merged[k].wait_value < w.wait_value:
                            merged[k] = w
                    else:
                        passthrough.append(w)
                kept = list(passthrough)
                for k, w in merged.items():
                    if dominance and es.get(k, -1) >= w.wait_value:
                        n += 1
                        continue
                    kept.append(w)
                    if dominance:
                        es[k] = w.wait_value
                if len(kept) != len(si.on_wait):
                    inst.sync_info = bass_rust.SyncInfo(
                        on_wait=kept, on_update=list(si.on_update)
                    )
    return n


def _split_excess_waits(nc, max_waits=1):
    """walrus in this container rejects >1 semaphore wait per instruction
    ("Too many sync wait commands"). Move excess waits onto preceding NoOps on
    the same engine; in-order engines block identically."""
    n = 0
    for f in nc.m.functions:
        for bb in f.blocks:
            insts = bb.instructions
            i = 0
            while i < len(insts):
                inst = insts[i]
                si = inst.sync_info
                if si is not None and len(si.on_wait) > max_waits:
                    waits = list(si.on_wait)
                    keep, excess = waits[-max_waits:], waits[:-max_waits]
                    nops = []
                    for w in excess:
                        _counter[0] += 1
                        nop = mybir.InstNoOp(
                            name=f"wsplit_{_counter[0]}", engine=inst.engine
                        )
                        nop.sync_info = bass_rust.SyncInfo(on_wait=[w], on_update=[])
                        nops.append(nop)
                    inst.sync_info = bass_rust.SyncInfo(
                        on_wait=keep, on_update=list(si.on_update)
                    )
                    insts[i:i] = nops
                    i += len(nops)
                    n += 1
                i += 1
    return n


def build_program(T, BL):
    """Build the SPMD Bass program for seq length T and BL batch rows/core."""
    assert T % GS == 0 and BL == 32
    groups = T // GS
    n_tok = T * BL  # tokens per core
    chunks = (GS * BL) // 128  # 128-row gather chunks per group (=4)

    nc = bass.Bass()

    tok_d = nc.declare_dram_parameter("tok", [128, chunks * groups], I32, isOutput=False)
    emb_d = nc.declare_dram_parameter("emb", [VOCAB, E], BF16, isOutput=False)
    w_d = nc.declare_dram_parameter("w", [128, 512], BF16, isOutput=False)
    b1_d = nc.declare_dram_parameter("b1", [128, 1], F32, isOutput=False)
    b2_d = nc.declare_dram_parameter("b2", [128, 1], F32, isOutput=False)
    clsw_d = nc.declare_dram_parameter("clsw", [128, 1], BF16, isOutput=False)
    clsb_d = nc.declare_dram_parameter("clsb", [1, 1], F32, isOutput=False)
    eqm_d = nc.declare_dram_parameter("eqm", [128, n_tok], BF16, isOutput=False)
    out_d = nc.declare_dram_parameter("out", [1, BL], F32, isOutput=True)

    with tile.TileContext(nc) as tc:
        with (
            tc.tile_pool(name="const", bufs=1) as const,
            tc.tile_pool(name="exg", bufs=3) as exgp,
            tc.tile_pool(name="ext", bufs=2) as extp,
            tc.tile_pool(name="mk", bufs=3) as mkp,
            tc.tile_pool(name="tmpp", bufs=2) as tmpp,
            tc.tile_pool(name="p1", bufs=2, space="PSUM") as p1p,
            tc.tile_pool(name="p2", bufs=2, space="PSUM") as p2p,
            tc.tile_pool(name="tp", bufs=2, space="PSUM") as tpp,
            tc.tile_pool(name="pc", bufs=1, space="PSUM") as pcp,
        ):
            # ---- persistent tiles ----
            tok_sb = const.tile([128, chunks * groups], I32)
            w_sb = const.tile([128, 512], BF16)
            b1_sb = const.tile([128, 1], F32)
            b2_sb = const.tile([128, 1], F32)
            clsw_sb = const.tile([128, 1], BF16)
            clsb_sb = const.tile([1, 1], F32)
            ident = const.tile([128, 128], BF16)
            h1h = const.tile([128, (T + 1) * BL], BF16)  # block t+1 = h1[t]
            h2r = const.tile([128, RING * BL], BF16)  # slot t%RING = h2[t]
            cap = const.tile([128, BL], BF16)
            capf = const.tile([128, BL], F32)
            redh = const.tile([128, BL * groups], F32)
            osb = const.tile([1, BL], F32)

            nc.sync.dma_start(out=tok_sb[:], in_=tok_d[:])
            nc.sync.dma_start(out=w_sb[:], in_=w_d[:])
            nc.sync.dma_start(out=b1_sb[:], in_=b1_d[:])
            nc.sync.dma_start(out=b2_sb[:], in_=b2_d[:])
            nc.sync.dma_start(out=clsw_sb[:], in_=clsw_d[:])
            nc.sync.dma_start(out=clsb_sb[:], in_=clsb_d[:])
            make_identity(nc, ident[:])
            nc.gpsimd.memset(h1h[:, 0:BL], 0.0)
            nc.gpsimd.memset(h2r[:, (RING - 1) * BL : RING * BL], 0.0)

            W_IH1 = w_sb[:, 0:128]
            W_HH1 = w_sb[:, 128:256]
            W_IH2 = w_sb[:, 256:384]
            W_HH2 = w_sb[:, 384:512]

            def h1_at(t):  # h1 state after step t; t=-1 is the zero block
                return h1h[:, (t + 1) * BL : (t + 2) * BL]

            def h2_at(t):  # h2 state after step t; t=-1 is ring slot RING-1
                s = t % RING
                return h2r[:, s * BL : (s + 1) * BL]

            ex_tiles = {}
            exT_tiles = {}
            mk_tiles = {}
            p1_tiles = {}
            p2_tiles = {}

            def emit_gather(g):
                exg = exgp.tile([128, GS * BL], BF16, name="exg")
                for j in range(chunks):
                    nc.gpsimd.indirect_dma_start(
                        out=exg[:, 128 * j : 128 * (j + 1)],
                        out_offset=None,
                        in_=emb_d[:, :],
                        in_offset=IndirectOffsetOnAxis(
                            ap=tok_sb[:, chunks * g + j : chunks * g + j + 1], axis=0
                        ),
                    )
                ex_tiles[g] = exg
                mk = mkp.tile([128, GS * BL], BF16, name="mk")
                nc.sync.dma_start(
                    out=mk[:], in_=eqm_d[:, GS * BL * g : GS * BL * (g + 1)]
                )
                mk_tiles[g] = mk

            def emit_transposes(g, lo, hi):
                if g not in exT_tiles:
                    # PSUM staging tile created alongside first transpose pair
                    tp_t = tpp.tile([128, GS * BL], BF16, space="PSUM", name="tp_t")
                    exT_t = extp.tile([128, GS * BL], BF16, name="exT_t")
                    exT_tiles[g] = (tp_t, exT_t)
                tp, _ = exT_tiles[g]
                exg = ex_tiles[g]
                for j in range(lo, hi):
                    sl = slice(128 * j, 128 * (j + 1))
                    nc.tensor.transpose(out=tp[:, sl], in_=exg[:, sl], identity=ident[:])

            def emit_ext_copy(g):
                tp, exT = exT_tiles[g]
                nc.vector.tensor_copy(out=exT[:], in_=tp[:])
                del ex_tiles[g]

            def emit_proj1(g):
                _, exT = exT_tiles.pop(g)
                p1 = p1p.tile([128, GS * BL], F32, space="PSUM", name="p1")
                nc.tensor.matmul(
                    p1[:], lhsT=W_IH1, rhs=exT[:], start=True, stop=False,
                    skip_group_check=True,
                )
                p1_tiles[g] = p1

            def emit_proj2(g):
                p2 = p2p.tile([128, GS * BL], F32, space="PSUM", name="p2")
                nc.tensor.matmul(
                    p2[:], lhsT=W_IH2,
                    rhs=h1h[:, (GS * g + 1) * BL : (GS * (g + 1) + 1) * BL],
                    start=True, stop=False, skip_group_check=True,
                )
                p2_tiles[g] = p2

            def emit_m1(t):
                p1 = p1_tiles[t // GS]
                s = t % GS
                nc.tensor.matmul(
                    p1[:, BL * s : BL * (s + 1)], lhsT=W_HH1, rhs=h1_at(t - 1),
                    start=False, stop=True, skip_group_check=True,
                )

            def emit_a1(t):
                p1 = p1_tiles[t // GS]
                s = t % GS
                nc.scalar.activation(
                    out=h1_at(t), in_=p1[:, BL * s : BL * (s + 1)],
                    func=mybir.ActivationFunctionType.Tanh, bias=b1_sb[:, 0:1],
                )

            def emit_m2(t):
                p2 = p2_tiles[t // GS]
                s = t % GS
                nc.tensor.matmul(
                    p2[:, BL * s : BL * (s + 1)], lhsT=W_HH2, rhs=h2_at(t - 1),
                    start=False, stop=True, skip_group_check=True,
                )

            def emit_a2(t):
                p2 = p2_tiles[t // GS]
                s = t % GS
                nc.scalar.activation(
                    out=h2_at(t), in_=p2[:, BL * s : BL * (s + 1)],
                    func=mybir.ActivationFunctionType.Tanh, bias=b2_sb[:, 0:1],
                )

            def emit_capture(g):
                mk = mk_tiles.pop(g)
                base = (g % (RING // GS)) * GS * BL
                tmp = tmpp.tile([128, GS * BL], BF16, name="cm")
                nc.vector.tensor_mul(tmp[:], h2r[:, base : base + GS * BL], mk[:])
                nc.vector.tensor_reduce(
                    out=redh[:, BL * g : BL * (g + 1)],
                    in_=tmp[:].rearrange("p (s b) -> p b s", s=GS, b=BL),
                    axis=mybir.AxisListType.X,
                    op=mybir.AluOpType.add,
                )
                del p2_tiles[g]

            # ---- prologue: stage group 0 ----
            emit_gather(0)
            emit_transposes(0, 0, chunks)
            emit_ext_copy(0)
            emit_proj1(0)

            # ---- main loop: L1 on group g, L2 on group g-1, prefetch g+1 ----
            for g in range(groups + 1):
                for s in range(GS):
                    t = GS * g + s
                    if g < groups:
                        emit_m1(t)
                    if s == 0 and g >= 1:
                        emit_proj2(g - 1)
                    if g >= 1:
                        emit_m2(GS * (g - 1) + s)
                    if g < groups:
                        emit_a1(t)
                    if g >= 1:
                        emit_a2(GS * (g - 1) + s)
                    if g + 1 < groups:
                        if s == 1:
                            emit_gather(g + 1)
                        elif s == 5:
                            emit_transposes(g + 1, 0, 2)
                        elif s == 6:
                            emit_transposes(g + 1, 2, chunks)
                        elif s == 7:
                            emit_ext_copy(g + 1)
                        elif s == 9:
                            emit_proj1(g + 1)
                if g >= 1:
                    emit_capture(g - 1)

            # fold per-group partial captures, then classifier
            nc.vector.tensor_reduce(
                out=capf[:],
                in_=redh[:].rearrange("p (g b) -> p b g", g=groups, b=BL),
                axis=mybir.AxisListType.X,
                op=mybir.AluOpType.add,
            )
            nc.vector.tensor_copy(out=cap[:], in_=capf[:])

            # classifier: logits[1, BL] = cls_w.T @ cap ; sigmoid
            pc = pcp.tile([1, BL], F32, space="PSUM")
            nc.tensor.matmul(pc[:], lhsT=clsw_sb[:], rhs=cap[:], start=True, stop=True)
            nc.scalar.activation(
                out=osb[:], in_=pc[:],
                func=mybir.ActivationFunctionType.Sigmoid, bias=clsb_sb[:, 0:1],
            )
            nc.sync.dma_start(out=out_d[:], in_=osb[:])

    import os
    # 0: none; 1: dominance only; 2: dominance + strip ACT same-engine waits;
    # 3: strip only (no dominance)
    mode = int(os.environ.get("KV2_ELIDE", "3"))
    if mode in (1, 2):
        _elide_redundant_waits(nc, strip_same_engine=(mode == 2))
    elif mode == 3:
        _elide_redundant_waits(nc, strip_same_engine=True, dominance=False)
    _split_excess_waits(nc)
    return nc


def make_core_inputs(x_c, lengths_c, emb_bf, w_pack, b1, b2, clsw_bf, clsb, T, BL):
    """Host-side prep of one core's input map. x_c [BL, T] int, lengths_c [BL]."""
    groups = T // GS
    chunks = (GS * BL) // 128
    # token index layout: flat[g, i] with i = t_loc*32 + b; chunk j = i >> 7
    A = x_c.T.reshape(groups, GS, BL).reshape(groups, GS * BL)  # [g, i]
    tok = (
        A.reshape(groups, chunks, 128).transpose(2, 0, 1).reshape(128, groups * chunks)
    )
    # column order must be [g*chunks + j]
    tok = np.ascontiguousarray(tok).astype(np.int32)
    eq = (np.arange(T)[:, None] == (lengths_c - 1)[None, :]).astype(NP_BF16)  # [T, BL]
    eqm = np.broadcast_to(eq.reshape(1, T * BL), (128, T * BL))
    eqm = np.ascontiguousarray(eqm)
    return {
        "tok": tok,
        "emb": emb_bf,
        "w": w_pack,
        "b1": b1,
        "b2": b2,
        "clsw": clsw_bf,
        "clsb": clsb,
        "eqm": eqm,
    }


def prep_in_maps(np_inputs, T, BL):
    x = np.asarray(np_inputs["x"])
    lengths = np.asarray(np_inputs["lengths"])
    emb = np_inputs["emb"]
    W_ih, W_hh, b = np_inputs["W_ih"], np_inputs["W_hh"], np_inputs["b"]
    cls_w, cls_b = np_inputs["cls_w"], np_inputs["cls_b"]
    emb_bf = np.asarray(emb, np.float32).astype(NP_BF16)
    w_pack = np.concatenate(
        [W_ih[0], W_hh[0], W_ih[1], W_hh[1]], axis=1
    ).astype(NP_BF16)
    b1 = np.asarray(b[0], np.float32).reshape(128, 1)
    b2 = np.asarray(b[1], np.float32).reshape(128, 1)
    clsw_bf = np.asarray(cls_w, np.float32).astype(NP_BF16).reshape(128, 1)
    clsb = np.asarray(cls_b, np.float32).reshape(1, 1)

    in_maps = []
    for c in range(N_CORES):
        rc = slice(c * BL, (c + 1) * BL)
        in_maps.append(
            make_core_inputs(
                x[rc].astype(np.int64),
                lengths[rc].astype(np.int64),
                emb_bf, w_pack, b1, b2, clsw_bf, clsb, T, BL,
            )
        )
    return in_maps


def run(x, lengths, emb, W_ih, W_hh, b, cls_w, cls_b, T, BL, trace=False):
    x = np.asarray(x)
    B = x.shape[0]
    assert B == N_CORES * BL and x.shape[1] == T
    in_maps = prep_in_maps(
        dict(x=x, lengths=lengths, emb=emb, W_ih=W_ih, W_hh=W_hh, b=b,
             cls_w=cls_w, cls_b=cls_b),
        T, BL,
    )

    import time as _time

    _t = _time.time()
    nc = build_program(T, BL)
    print(f"[kernel] build_program: {_time.time() - _t:.1f}s", flush=True)
    _t = _time.time()
    res = run_bass_kernel_spmd(
        nc,
        in_maps,
        list(range(N_CORES)),
        trace=trace,
        trace_cores=list(range(N_CORES)) if trace else None,
    )
    print(f"[kernel] compile+exec: {_time.time() - _t:.1f}s", flush=True)
    out = np.concatenate(
        [res.results[c]["out"].reshape(BL) for c in range(N_CORES)]
    ).reshape(B, 1).astype(np.float32)
    return out, res


def kernel(x, lengths, emb, W_ih, W_hh, b, cls_w, cls_b):
    out, _ = run(x, lengths, emb, W_ih, W_hh, b, cls_w, cls_b, T=2048, BL=32)
    return out
